# revision 13
# baseline (speedup 1.0000x reference)
"""Causal self-attention (flipped mask: attend to k >= q) on 8 Trainium2 cores.

Sharding: 2-way data parallel over batch x 4-way head parallel (4 heads/core).
Each core computes x[b] -> qkv (its 4 heads) -> attention -> partial out-proj
(its 256 rows of Wo); the host sums the 4 partials per batch and adds the
out-proj bias there.

v5 structure (per core):
  - x transposed on the HOST; xT [C, T] f16 DMA'd to SBUF; slab 0 split
    per c-chunk so the first chain starts after ~128KB.
  - phase B chains m-major (k-g0, k-g1, q-g0 per t-slab, q-g1 last) so each
    slab is consumed right as it lands and kT finishes early; qk PSUM
    evacuation on DVE (scalar_tensor_tensor; SCALE folded into wq on host)
    keeping ACT free; the (g0,n0) scores+exps prefill under the B tail.
  - scores of a head pair in ONE [128,1024] 2-bank PSUM tile; exp is ONE
    ACT instruction per k-tile; no additive mask (0/1 triangular f16
    multiply + gpsimd memset of the masked strip; band exp width trimmed).
  - after B: the (0,0) AV batch interleaves with (1,0)'s score pairs so
    neither PE nor ACT idles.
  - softmax denominator folded into AV via a ones column in v; for n<2 the
    reciprocal goes through the DMA-reshape trick (latency hidden); for the
    small tail groups (n>=2) it is computed as exp(-log(d)) on the
    otherwise-idle ACT engine - no DMA latency on the critical path.
  - phase D per q-chunk, emitted one group late so it never waits on a
    normalize chain; evacuation on DVE early / ACT late; f16 out per t-tile.
"""

import numpy as np

B, T, C = 2, 2048, 1024
H = 16
D = 64
NH = 4           # heads per core
HC = NH * D      # 256 local head cols
SCALE = 0.125    # 1/sqrt(D)
N_CORES = 8

NT = T // 128    # 16 t-tiles
NCC = C // 128   # 8 c-chunks
NQ = T // 512    # 4 q-chunks of 512
NJ = T // 128    # 16 kt-chunks of 128

_CACHE = {}


def _build_nc():
    import concourse.tile as tile
    from concourse import bacc, mybir

    f32 = mybir.dt.float32
    f16 = mybir.dt.float16
    Exp = mybir.ActivationFunctionType.Exp
    Ln = mybir.ActivationFunctionType.Ln
    Ident = mybir.ActivationFunctionType.Identity
    Add = mybir.AluOpType.add
    Bypass = mybir.AluOpType.bypass

    nc = bacc.Bacc(None, target_bir_lowering=False, debug=False)

    xbT = nc.dram_tensor("xbT", [C, T], f16, kind="ExternalInput")
    wq = nc.dram_tensor("wq", [C, HC], f16, kind="ExternalInput")
    wk = nc.dram_tensor("wk", [C, HC], f16, kind="ExternalInput")
    wv = nc.dram_tensor("wv", [C, HC], f16, kind="ExternalInput")
    bqs = nc.dram_tensor("bqs", [HC], f32, kind="ExternalInput")
    bk = nc.dram_tensor("bk", [HC], f32, kind="ExternalInput")
    bvb = nc.dram_tensor("bvb", [128, HC], f32, kind="ExternalInput")
    wo = nc.dram_tensor("wo", [HC, C], f16, kind="ExternalInput")
    tri01 = nc.dram_tensor("tri01", [128, 128], f16, kind="ExternalInput")
    out = nc.dram_tensor("out", [T, C], f16, kind="ExternalOutput")

    with tile.TileContext(nc) as tc, (
        tc.tile_pool(name="consts", bufs=1)) as consts, (
        tc.tile_pool(name="wts", bufs=1)) as wts, (
        tc.tile_pool(name="persist", bufs=1)) as persist:

        # ---- weights needed at phase-B start ----
        wq_sb = wts.tile([128, NCC, HC], f16)
        nc.sync.dma_start(out=wq_sb, in_=wq.rearrange("(a p) n -> p a n", p=128))
        wk_sb = wts.tile([128, NCC, HC], f16)
        nc.sync.dma_start(out=wk_sb, in_=wk.rearrange("(a p) n -> p a n", p=128))
        wv_sb = wts.tile([128, NCC, HC], f16)
        nc.sync.dma_start(out=wv_sb, in_=wv.rearrange("(a p) n -> p a n", p=128))

        # ---- x in t-slabs, consumption order; slab 0 split per c-chunk ----
        xT_sb = persist.tile([128, NCC, T], f16)
        xTr = xbT.rearrange("(a p) t -> p a t", p=128)
        for c0 in range(NCC):
            nc.sync.dma_start(out=xT_sb[:, c0, 0:512], in_=xTr[:, c0, 0:512])
        for m in range(1, NQ):
            nc.sync.dma_start(
                out=xT_sb[:, :, m * 512:(m + 1) * 512],
                in_=xTr[:, :, m * 512:(m + 1) * 512],
            )

        # ---- small consts; wo last (phase D only) ----
        tri_sb = consts.tile([128, 128], f16)
        nc.sync.dma_start(out=tri_sb, in_=tri01[:, :])
        bq_sb = consts.tile([128, 2], f32)
        nc.sync.dma_start(out=bq_sb, in_=bqs.rearrange("(a p) -> p a", p=128))
        bk_sb = consts.tile([128, 2], f32)
        nc.sync.dma_start(out=bk_sb, in_=bk.rearrange("(a p) -> p a", p=128))
        bvb_sb = consts.tile([128, NH, D], f32)
        nc.sync.dma_start(out=bvb_sb, in_=bvb.rearrange("p (h d) -> p h d", h=NH))
        wo_sb = wts.tile([128, 2, C], f16)
        nc.sync.dma_start(out=wo_sb, in_=wo.rearrange("(a p) n -> p a n", p=128))

        # ---- persistent activations ----
        qT_sb = persist.tile([128, 2, T], f16)
        kT_sb = persist.tile([128, 2, T], f16)
        # v, augmented: per t-tile, per pair g: [65 even | 130 odd]
        # even block: cols 0..63 = v(2g), col 64 = 1.0
        # odd block:  col 0 = 1.0 (tile col 65), cols 64..127 = v(2g+1)
        v_sb = persist.tile([128, NT, 2, 195], f16)
        yT_sb = persist.tile([128, 2, T], f16)

        for t0 in range(NT):
            nc.gpsimd.memset(v_sb[:, t0, :, 64:66], 1.0)
        # cols 66:129 and 193:195 feed junk output partitions that are never
        # read; zero them anyway (keeps the race/uninit checkers clean)
        nc.gpsimd.memset(v_sb[:, :, :, 66:129], 0.0)
        nc.gpsimd.memset(v_sb[:, :, :, 193:195], 0.0)

        with (
            tc.tile_pool(name="epool", bufs=20) as epool,
            tc.tile_pool(name="rpool", bufs=2) as rpool,
            tc.tile_pool(name="opool", bufs=2) as opool,
            tc.tile_pool(name="psS", bufs=2, space="PSUM") as psS,
        ):
            def emit_score_exp(g, n, j):
                qs = n * 512
                b_i = j - 4 * n
                ks = j * 128
                ps = psS.tile([128, 1024], f32, tag="s", name="ps")
                nc.tensor.matmul(
                    ps[:, 0:512],
                    lhsT=(kT_sb[0:64, g, ks:ks + 128]),
                    rhs=(qT_sb[0:64, g, qs:qs + 512]),
                    start=True, stop=True,
                )
                nc.tensor.matmul(
                    ps[:, 512:1024],
                    lhsT=(kT_sb[64:128, g, ks:ks + 128]),
                    rhs=(qT_sb[64:128, g, qs:qs + 512]),
                    start=True, stop=True,
                )
                e = epool.tile([128, 1024], f16, tag="e", name="e")
                if b_i < 4:
                    w = 128 * (b_i + 1)
                    e3 = e.rearrange("p (h q) -> p h q", h=2)
                    ps3 = ps.rearrange("p (h q) -> p h q", h=2)
                    nc.scalar.activation(e3[:, :, 0:w], ps3[:, :, 0:w], Exp)
                    nc.vector.tensor_mul(e[:, w - 128:w], e[:, w - 128:w], tri_sb)
                    nc.vector.tensor_mul(
                        e[:, 512 + w - 128:512 + w], e[:, 512 + w - 128:512 + w],
                        tri_sb,
                    )
                    if w < 512:
                        nc.gpsimd.memset(e3[:, :, w:512], 0.0)
                else:
                    nc.scalar.activation(e, ps, Exp)
                return e

            def emit_av(g, n, j, e, yt):
                nc.tensor.matmul(
                    yt[0:65, 0:512],
                    lhsT=(v_sb[:, j, g, 0:65]),
                    rhs=(e[:, 0:512]),
                    start=(j == 4 * n), stop=(j == NJ - 1),
                )
                nc.tensor.matmul(
                    yt[:, 512:1024],
                    lhsT=(v_sb[:, j, g, 65:193]),
                    rhs=(e[:, 512:1024]),
                    start=(j == 4 * n), stop=(j == NJ - 1),
                )

            def emit_normalize(g, n, yt, act_recip):
                # denominators: even head @ psum partition 64 of the low
                # bank, odd head @ partition 0 of the high bank
                ye = yt[:, 0:512]
                yo = yt[:, 512:1024]
                qs = n * 512
                bsbE = rpool.tile([128, 512], f32, tag="bsbE", name="bsbE")
                bsbO = rpool.tile([128, 512], f32, tag="bsbO", name="bsbO")
                if act_recip:
                    # 1/d = exp(-ln d) on ACT reading the PSUM rows directly;
                    # used for the small tail groups where ACT is otherwise
                    # idle.  NOTE: partition_broadcast silently reads garbage
                    # for a non-zero base partition on HW, so the even-head
                    # row (partition 64) is DMA-moved to partition 0 first.
                    rcp = rpool.tile([128, 512], f32, tag="rcp", name="rcp")
                    rcz = rpool.tile([128, 512], f32, tag="rcz", name="rcz")
                    nc.scalar.activation(rcp[64:65, :], ye[64:65, :], Ln)
                    nc.scalar.activation(rcp[64:65, :], rcp[64:65, :], Exp,
                                         scale=-1.0)
                    nc.scalar.activation(rcp[0:1, :], yo[0:1, :], Ln)
                    nc.scalar.activation(rcp[0:1, :], rcp[0:1, :], Exp,
                                         scale=-1.0)
                    nc.sync.dma_start(out=rcz[0:1, :], in_=rcp[64:65, :])
                    nc.gpsimd.partition_broadcast(bsbE[:, :], rcz[0:1, :])
                    nc.gpsimd.partition_broadcast(bsbO[:, :], rcp[0:1, :])
                else:
                    tmp = rpool.tile([128, 512], f32, tag="tmp", name="tmp")
                    nc.vector.tensor_copy(tmp[64:65, :], ye[64:65, :])
                    nc.vector.tensor_copy(tmp[0:1, :], yo[0:1, :])
                    # DMA-reshape [1,512]->[128,4] so the iterative DVE
                    # reciprocal runs on a tiny free dim, then DMA back
                    rs = rpool.tile([128, 8], f32, tag="rs", name="rs")
                    nc.sync.dma_start(out=rs[:, 0:4], in_=tmp[64:65, :])
                    nc.sync.dma_start(out=rs[:, 4:8], in_=tmp[0:1, :])
                    rr = rpool.tile([128, 8], f32, tag="rr", name="rr")
                    nc.vector.reciprocal(rr, rs)
                    rt = rpool.tile([128, 1024], f32, tag="rt", name="rt")
                    nc.sync.dma_start(out=rt[0:1, 0:512], in_=rr[:, 0:4])
                    nc.sync.dma_start(out=rt[0:1, 512:1024], in_=rr[:, 4:8])
                    nc.gpsimd.partition_broadcast(bsbE[:, :], rt[0:1, 0:512])
                    nc.gpsimd.partition_broadcast(bsbO[:, :], rt[0:1, 512:1024])
                nc.vector.tensor_mul(
                    yT_sb[0:64, g, qs:qs + 512], ye[0:64, :], bsbE[0:64, :]
                )
                nc.vector.tensor_mul(
                    yT_sb[64:128, g, qs:qs + 512], yo[64:128, :], bsbO[64:128, :]
                )

            # -------- phase B: projections, m-major --------
            chain_specs = []
            for m in range(NQ):
                chain_specs += [(0, m, 1), (1, m, 1), (0, m, 0)]
            chain_specs += [(1, m, 0) for m in range(NQ)]
            prefill = []   # (j, e) for (g=0, n=0); window opens at chain 10
            pf_quota = [0, 0, 0, 0, 0, 0, 0, 0, 0, 0, 3, 3, 3, 3, 2, 2]

            with tc.tile_pool(name="psB", bufs=4, space="PSUM") as psB:
                for i, (g, m, is_k) in enumerate(chain_specs):
                    w_sb = wk_sb if is_k else wq_sb
                    t0 = i
                    psqk = psB.tile([128, 512], f32, tag="pj", name="psqk")
                    psv = psB.tile([128, 512], f32, tag="pj", name="psv")
                    for c0 in range(NCC):
                        nc.tensor.matmul(
                            psqk,
                            lhsT=(w_sb[:, c0, g * 128:(g + 1) * 128]),
                            rhs=(xT_sb[:, c0, m * 512:(m + 1) * 512]),
                            start=(c0 == 0), stop=(c0 == NCC - 1),
                        )
                        nc.tensor.matmul(
                            psv[:, 0:HC],
                            lhsT=(xT_sb[:, c0, t0 * 128:(t0 + 1) * 128]),
                            rhs=(wv_sb[:, c0, :]),
                            start=(c0 == 0), stop=(c0 == NCC - 1),
                        )
                    # qk evacuation on DVE (ACT stays free for prefill exps);
                    # SCALE is folded into wq on the host
                    dst = kT_sb if is_k else qT_sb
                    bias = bk_sb if is_k else bq_sb
                    nc.vector.tensor_scalar_add(
                        dst[:, g, m * 512:(m + 1) * 512], psqk,
                        bias[:, g:g + 1],
                    )
                    psv4 = psv[:, 0:HC].rearrange("p (h d) -> p h d", h=NH)
                    for gg in range(2):
                        nc.vector.tensor_add(
                            v_sb[:, t0, gg, 0:64], psv4[:, 2 * gg, :],
                            bvb_sb[:, 2 * gg, :],
                        )
                        nc.vector.tensor_add(
                            v_sb[:, t0, gg, 129:193], psv4[:, 2 * gg + 1, :],
                            bvb_sb[:, 2 * gg + 1, :],
                        )
                    for _ in range(pf_quota[i]):
                        j = len(prefill)
                        prefill.append((j, emit_score_exp(0, 0, j)))

            # -------- phases C/D --------
            with tc.tile_pool(name="psY", bufs=2, space="PSUM") as psY:

                def run_group(g, n):
                    yt = psY.tile([128, 1024], f32, tag="y", name="yt")
                    lag = []
                    for j in range(4 * n, NJ):
                        e = emit_score_exp(g, n, j)
                        if len(lag) >= 2:
                            emit_av(g, n, *lag.pop(0), yt)
                        lag.append((j, e))
                    for item in lag:
                        emit_av(g, n, *item, yt)
                    emit_normalize(g, n, yt, act_recip=(n >= 2))

                def emit_d(n):
                    for t0 in range(4 * n, 4 * n + 4):
                        o_sb = opool.tile([128, C], f16, tag="o", name="o_sb")
                        pd = psS.tile([128, 1024], f32, tag="s", name="pd")
                        for g in range(2):
                            nc.tensor.matmul(
                                pd[:, 0:512],
                                lhsT=(yT_sb[:, g, t0 * 128:(t0 + 1) * 128]),
                                rhs=(wo_sb[:, g, 0:512]),
                                start=(g == 0), stop=(g == 1),
                            )
                            nc.tensor.matmul(
                                pd[:, 512:1024],
                                lhsT=(yT_sb[:, g, t0 * 128:(t0 + 1) * 128]),
                                rhs=(wo_sb[:, g, 512:1024]),
                                start=(g == 0), stop=(g == 1),
                            )
                        if n < 2:
                            nc.vector.tensor_copy(o_sb, pd)
                        else:
                            nc.scalar.activation(o_sb, pd, Ident)
                        nc.sync.dma_start(
                            out=out[t0 * 128:(t0 + 1) * 128, :], in_=o_sb
                        )

                # n=0: (0,0) AV batch interleaved with (1,0) score pairs so
                # PE and ACT both stay fed right after phase B
                yt00 = psY.tile([128, 1024], f32, tag="y", name="yt")
                yt10 = psY.tile([128, 1024], f32, tag="y", name="yt")
                lag10 = []
                for j in range(NJ):
                    e10 = emit_score_exp(1, 0, j)
                    emit_av(0, 0, j, prefill[j][1], yt00)
                    if len(lag10) >= 2:
                        emit_av(1, 0, *lag10.pop(0), yt10)
                    lag10.append((j, e10))
                for item in lag10:
                    emit_av(1, 0, *item, yt10)
                emit_normalize(0, 0, yt00, act_recip=False)
                emit_normalize(1, 0, yt10, act_recip=False)

                for n in range(1, NQ):
                    run_group(0, n)
                    emit_d(n - 1)
                    run_group(1, n)
                emit_d(NQ - 1)

    nc.compile()
    return nc


def _host_consts():
    p = np.arange(128)[:, None]
    c = np.arange(128)[None, :]
    tri01 = (p >= c).astype(np.float16)
    return tri01


def make_in_maps(x, Wqkv, bqkv, Wo, bo):
    x = np.asarray(x, dtype=np.float32)
    Wqkv = np.asarray(Wqkv, dtype=np.float32)
    bqkv = np.asarray(bqkv, dtype=np.float32)
    Wo = np.asarray(Wo, dtype=np.float32)
    tri01 = _host_consts()
    xT = [np.ascontiguousarray(x[b].T).astype(np.float16) for b in range(B)]
    in_maps = []
    for core in range(N_CORES):
        b, hg = divmod(core, 4)
        s = HC * hg
        in_maps.append({
            "xbT": xT[b],
            # SCALE folded into wq (and bqs) so the q evacuation is a plain
            # bias add on DVE
            "wq": (np.ascontiguousarray(Wqkv[:, s:s + HC])
                   * np.float32(SCALE)).astype(np.float16),
            "wk": np.ascontiguousarray(Wqkv[:, C + s:C + s + HC]).astype(np.float16),
            "wv": np.ascontiguousarray(Wqkv[:, 2 * C + s:2 * C + s + HC]).astype(np.float16),
            "bqs": np.ascontiguousarray(bqkv[s:s + HC]) * np.float32(SCALE),
            "bk": np.ascontiguousarray(bqkv[C + s:C + s + HC]),
            "bvb": np.ascontiguousarray(
                np.broadcast_to(bqkv[2 * C + s:2 * C + s + HC], (128, HC))
            ),
            "wo": np.ascontiguousarray(Wo[s:s + HC, :]).astype(np.float16),
            "tri01": tri01,
        })
    return in_maps


def unshard(results, bo=None):
    out = np.empty((B, T, C), dtype=np.float32)
    for b in range(B):
        acc = results[4 * b]["out"].astype(np.float32)
        for hg in range(1, 4):
            acc = acc + results[4 * b + hg]["out"].astype(np.float32)
        if bo is not None:
            acc = acc + np.asarray(bo, dtype=np.float32)
        out[b] = acc
    return out


def get_nc():
    if "nc" not in _CACHE:
        _CACHE["nc"] = _build_nc()
    return _CACHE["nc"]


def kernel(x, Wqkv, bqkv, Wo, bo):
    from concourse.bass_utils import run_bass_kernel_spmd

    nc = get_nc()
    in_maps = make_in_maps(x, Wqkv, bqkv, Wo, bo)
    res = run_bass_kernel_spmd(nc, in_maps, list(range(N_CORES)))
    return unshard(res.results, bo=bo)


# revision 14
# speedup vs baseline: 1.0429x; 1.0429x over previous
"""Causal self-attention (flipped mask: attend to k >= q) on 8 Trainium2 cores.

Sharding: 2-way data parallel over batch x 4-way head parallel (4 heads/core).
Each core computes x[b] -> qkv (its 4 heads) -> attention -> partial out-proj
(its 256 rows of Wo); the host sums the 4 partials per batch and adds the
out-proj bias there.

v6 structure (per core):
  - x transposed on the HOST; xT [C, T] f16 DMA'd to SBUF as 8 c-chunk
    transfers covering full T (4KB contiguous lines; the first projection
    chain paces along chunk arrivals, later chains never stall).
  - phase B chains m-major (k-g0, k-g1, q-g0 per t-window, q-g1 last) so
    kT(g0) and early qT(g0) finish soon; qk PSUM evacuation on DVE
    (SCALE folded into wq on host) keeping ACT free for softmax exps.
  - attention q-chunks processed in DESCENDING size order (n=3,2,1,0): the
    small tail groups run right after B (their scores+exps prefill under
    the B tail), and every normalize chain + out-proj block hides under a
    following larger group.  Groups (0,3) and (0,2) are fully prescored in
    B; their AV batches interleave with the sibling group's score pairs so
    PE and ACT stay fed.
  - scores of a head pair in ONE [128,1024] 2-bank PSUM tile; exp is ONE
    ACT instruction per k-tile; no additive mask (0/1 triangular f16
    multiply + gpsimd memset of the masked strip; band exp width trimmed).
  - softmax denominator folded into AV via a ones column in v; reciprocal
    via the DMA-reshape trick (all latency hidden by group ordering).
  - phase D per q-chunk, one group delayed, PSUM->SBUF f16 copy on DVE,
    per-t-tile f16 DMA out.
"""

import numpy as np

B, T, C = 2, 2048, 1024
H = 16
D = 64
NH = 4           # heads per core
HC = NH * D      # 256 local head cols
SCALE = 0.125    # 1/sqrt(D)
N_CORES = 8

NT = T // 128    # 16 t-tiles
NCC = C // 128   # 8 c-chunks
NQ = T // 512    # 4 q-chunks of 512
NJ = T // 128    # 16 kt-chunks of 128

_CACHE = {}


def _build_nc():
    import concourse.tile as tile
    from concourse import bacc, mybir

    f32 = mybir.dt.float32
    f16 = mybir.dt.float16
    Exp = mybir.ActivationFunctionType.Exp

    nc = bacc.Bacc(None, target_bir_lowering=False, debug=False)

    xbT = nc.dram_tensor("xbT", [C, T], f16, kind="ExternalInput")
    wq = nc.dram_tensor("wq", [C, HC], f16, kind="ExternalInput")
    wk = nc.dram_tensor("wk", [C, HC], f16, kind="ExternalInput")
    wv = nc.dram_tensor("wv", [C, HC], f16, kind="ExternalInput")
    bqs = nc.dram_tensor("bqs", [HC], f32, kind="ExternalInput")
    bk = nc.dram_tensor("bk", [HC], f32, kind="ExternalInput")
    bvb = nc.dram_tensor("bvb", [128, HC], f32, kind="ExternalInput")
    wo = nc.dram_tensor("wo", [HC, C], f16, kind="ExternalInput")
    tri01 = nc.dram_tensor("tri01", [128, 128], f16, kind="ExternalInput")
    out = nc.dram_tensor("out", [T, C], f16, kind="ExternalOutput")

    with tile.TileContext(nc) as tc, (
        tc.tile_pool(name="consts", bufs=1)) as consts, (
        tc.tile_pool(name="wts", bufs=1)) as wts, (
        tc.tile_pool(name="persist", bufs=1)) as persist:

        # ---- weights needed at phase-B start ----
        wk_sb = wts.tile([128, NCC, HC], f16)
        nc.sync.dma_start(out=wk_sb, in_=wk.rearrange("(a p) n -> p a n", p=128))
        wq_sb = wts.tile([128, NCC, HC], f16)
        nc.sync.dma_start(out=wq_sb, in_=wq.rearrange("(a p) n -> p a n", p=128))
        wv_sb = wts.tile([128, NCC, HC], f16)
        nc.sync.dma_start(out=wv_sb, in_=wv.rearrange("(a p) n -> p a n", p=128))

        # ---- x as 8 c-chunk DMAs over full T (4KB contiguous lines) ----
        xT_sb = persist.tile([128, NCC, T], f16)
        xTr = xbT.rearrange("(a p) t -> p a t", p=128)
        for c0 in range(NCC):
            nc.sync.dma_start(out=xT_sb[:, c0, :], in_=xTr[:, c0, :])

        # ---- small consts; wo last (phase D only) ----
        tri_sb = consts.tile([128, 128], f16)
        nc.sync.dma_start(out=tri_sb, in_=tri01[:, :])
        bq_sb = consts.tile([128, 2], f32)
        nc.sync.dma_start(out=bq_sb, in_=bqs.rearrange("(a p) -> p a", p=128))
        bk_sb = consts.tile([128, 2], f32)
        nc.sync.dma_start(out=bk_sb, in_=bk.rearrange("(a p) -> p a", p=128))
        bvb_sb = consts.tile([128, NH, D], f32)
        nc.sync.dma_start(out=bvb_sb, in_=bvb.rearrange("p (h d) -> p h d", h=NH))
        wo_sb = wts.tile([128, 2, C], f16)
        nc.sync.dma_start(out=wo_sb, in_=wo.rearrange("(a p) n -> p a n", p=128))

        # ---- persistent activations ----
        qT_sb = persist.tile([128, 2, T], f16)
        kT_sb = persist.tile([128, 2, T], f16)
        # v, augmented: per t-tile, per pair g: [65 even | 130 odd]
        # even block: cols 0..63 = v(2g), col 64 = 1.0
        # odd block:  col 0 = 1.0 (tile col 65), cols 64..127 = v(2g+1)
        v_sb = persist.tile([128, NT, 2, 195], f16)
        yT_sb = persist.tile([128, 2, T], f16)

        for t0 in range(NT):
            nc.gpsimd.memset(v_sb[:, t0, :, 64:66], 1.0)
        # cols 66:129 / 193:195 feed junk output partitions (never read);
        # zeroed to keep race/uninit checkers clean
        nc.gpsimd.memset(v_sb[:, :, :, 66:129], 0.0)
        nc.gpsimd.memset(v_sb[:, :, :, 193:195], 0.0)

        with (
            tc.tile_pool(name="epool", bufs=20) as epool,
            tc.tile_pool(name="rpool", bufs=2) as rpool,
            tc.tile_pool(name="opool", bufs=2) as opool,
            tc.tile_pool(name="psS", bufs=2, space="PSUM") as psS,
        ):
            def emit_score_exp(g, n, j):
                qs = n * 512
                b_i = j - 4 * n
                ks = j * 128
                ps = psS.tile([128, 1024], f32, tag="s", name="ps")
                nc.tensor.matmul(
                    ps[:, 0:512],
                    lhsT=(kT_sb[0:64, g, ks:ks + 128]),
                    rhs=(qT_sb[0:64, g, qs:qs + 512]),
                    start=True, stop=True,
                )
                nc.tensor.matmul(
                    ps[:, 512:1024],
                    lhsT=(kT_sb[64:128, g, ks:ks + 128]),
                    rhs=(qT_sb[64:128, g, qs:qs + 512]),
                    start=True, stop=True,
                )
                e = epool.tile([128, 1024], f16, tag="e", name="e")
                if b_i < 4:
                    w = 128 * (b_i + 1)
                    e3 = e.rearrange("p (h q) -> p h q", h=2)
                    ps3 = ps.rearrange("p (h q) -> p h q", h=2)
                    nc.scalar.activation(e3[:, :, 0:w], ps3[:, :, 0:w], Exp)
                    nc.vector.tensor_mul(e[:, w - 128:w], e[:, w - 128:w], tri_sb)
                    nc.vector.tensor_mul(
                        e[:, 512 + w - 128:512 + w], e[:, 512 + w - 128:512 + w],
                        tri_sb,
                    )
                    if w < 512:
                        nc.gpsimd.memset(e3[:, :, w:512], 0.0)
                else:
                    nc.scalar.activation(e, ps, Exp)
                return e

            def emit_av(g, n, j, e, yt):
                nc.tensor.matmul(
                    yt[0:65, 0:512],
                    lhsT=(v_sb[:, j, g, 0:65]),
                    rhs=(e[:, 0:512]),
                    start=(j == 4 * n), stop=(j == NJ - 1),
                )
                nc.tensor.matmul(
                    yt[:, 512:1024],
                    lhsT=(v_sb[:, j, g, 65:193]),
                    rhs=(e[:, 512:1024]),
                    start=(j == 4 * n), stop=(j == NJ - 1),
                )

            def emit_normalize(g, n, yt):
                # denominators: even head @ psum partition 64 of the low
                # bank, odd head @ partition 0 of the high bank.  DMA-reshape
                # [1,512]->[128,4] so the iterative DVE reciprocal runs on a
                # tiny free dim, then DMA + partition-broadcast 1/sum back.
                # (partition_broadcast only works from base partition 0!)
                ye = yt[:, 0:512]
                yo = yt[:, 512:1024]
                qs = n * 512
                tmp = rpool.tile([128, 512], f32, tag="tmp", name="tmp")
                nc.vector.tensor_copy(tmp[64:65, :], ye[64:65, :])
                nc.vector.tensor_copy(tmp[0:1, :], yo[0:1, :])
                rs = rpool.tile([128, 8], f32, tag="rs", name="rs")
                nc.sync.dma_start(out=rs[:, 0:4], in_=tmp[64:65, :])
                nc.sync.dma_start(out=rs[:, 4:8], in_=tmp[0:1, :])
                rr = rpool.tile([128, 8], f32, tag="rr", name="rr")
                nc.vector.reciprocal(rr, rs)
                rt = rpool.tile([128, 1024], f32, tag="rt", name="rt")
                nc.sync.dma_start(out=rt[0:1, 0:512], in_=rr[:, 0:4])
                nc.sync.dma_start(out=rt[0:1, 512:1024], in_=rr[:, 4:8])
                bsbE = rpool.tile([128, 512], f32, tag="bsbE", name="bsbE")
                bsbO = rpool.tile([128, 512], f32, tag="bsbO", name="bsbO")
                nc.gpsimd.partition_broadcast(bsbE[:, :], rt[0:1, 0:512])
                nc.gpsimd.partition_broadcast(bsbO[:, :], rt[0:1, 512:1024])
                nc.vector.tensor_mul(
                    yT_sb[0:64, g, qs:qs + 512], ye[0:64, :], bsbE[0:64, :]
                )
                nc.vector.tensor_mul(
                    yT_sb[64:128, g, qs:qs + 512], yo[64:128, :], bsbO[64:128, :]
                )

            # -------- phase B: projections, m-major; prefill of the small
            # first attention groups (g0,n3) and (g0,n2) under the B tail --
            chain_specs = []
            for m in range(NQ):
                chain_specs += [(0, m, 1), (1, m, 1), (0, m, 0)]
            chain_specs += [(1, m, 0) for m in range(NQ)]
            # (chain index) -> list of (g, n, j) score emissions after it;
            # kT(g0) complete after chain 9; qT(g0,m2) after chain 8,
            # qT(g0,m3) after chain 11
            pf_plan = {
                10: [(0, 2, 8), (0, 2, 9)],
                11: [(0, 2, 10), (0, 2, 11)],
                12: [(0, 2, 12), (0, 3, 12)],
                13: [(0, 2, 13), (0, 3, 13)],
                14: [(0, 2, 14), (0, 3, 14)],
                15: [(0, 2, 15), (0, 3, 15)],
            }
            pre = {(0, 2): {}, (0, 3): {}}

            with tc.tile_pool(name="psB", bufs=4, space="PSUM") as psB:
                for i, (g, m, is_k) in enumerate(chain_specs):
                    w_sb = wk_sb if is_k else wq_sb
                    t0 = i
                    psqk = psB.tile([128, 512], f32, tag="pj", name="psqk")
                    psv = psB.tile([128, 512], f32, tag="pj", name="psv")
                    for c0 in range(NCC):
                        nc.tensor.matmul(
                            psqk,
                            lhsT=(w_sb[:, c0, g * 128:(g + 1) * 128]),
                            rhs=(xT_sb[:, c0, m * 512:(m + 1) * 512]),
                            start=(c0 == 0), stop=(c0 == NCC - 1),
                        )
                        nc.tensor.matmul(
                            psv[:, 0:HC],
                            lhsT=(xT_sb[:, c0, t0 * 128:(t0 + 1) * 128]),
                            rhs=(wv_sb[:, c0, :]),
                            start=(c0 == 0), stop=(c0 == NCC - 1),
                        )
                    # qk evacuation on DVE; SCALE folded into wq on host
                    dst = kT_sb if is_k else qT_sb
                    bias = bk_sb if is_k else bq_sb
                    nc.vector.tensor_scalar_add(
                        dst[:, g, m * 512:(m + 1) * 512], psqk,
                        bias[:, g:g + 1],
                    )
                    psv4 = psv[:, 0:HC].rearrange("p (h d) -> p h d", h=NH)
                    for gg in range(2):
                        nc.vector.tensor_add(
                            v_sb[:, t0, gg, 0:64], psv4[:, 2 * gg, :],
                            bvb_sb[:, 2 * gg, :],
                        )
                        nc.vector.tensor_add(
                            v_sb[:, t0, gg, 129:193], psv4[:, 2 * gg + 1, :],
                            bvb_sb[:, 2 * gg + 1, :],
                        )
                    for (pg, pn, pj) in pf_plan.get(i, []):
                        pre[(pg, pn)][pj] = emit_score_exp(pg, pn, pj)

            # -------- phases C/D, descending q-chunk size --------
            with tc.tile_pool(name="psY", bufs=2, space="PSUM") as psY:

                def run_pair(n, pre_a):
                    # group (0,n) fully prescored; interleave its AV batch
                    # with (1,n)'s score pairs, then both normalizes
                    yta = psY.tile([128, 1024], f32, tag="y", name="yt")
                    ytb = psY.tile([128, 1024], f32, tag="y", name="yt")
                    lag = []
                    for j in range(4 * n, NJ):
                        e_b = emit_score_exp(1, n, j)
                        emit_av(0, n, j, pre_a[j], yta)
                        if len(lag) >= 2:
                            emit_av(1, n, *lag.pop(0), ytb)
                        lag.append((j, e_b))
                    emit_normalize(0, n, yta)
                    for item in lag:
                        emit_av(1, n, *item, ytb)
                    emit_normalize(1, n, ytb)

                def run_group(g, n):
                    yt = psY.tile([128, 1024], f32, tag="y", name="yt")
                    lag = []
                    for j in range(4 * n, NJ):
                        e = emit_score_exp(g, n, j)
                        if len(lag) >= 2:
                            emit_av(g, n, *lag.pop(0), yt)
                        lag.append((j, e))
                    for item in lag:
                        emit_av(g, n, *item, yt)
                    emit_normalize(g, n, yt)

                def emit_d(n):
                    for t0 in range(4 * n, 4 * n + 4):
                        o_sb = opool.tile([128, C], f16, tag="o", name="o_sb")
                        pd = psS.tile([128, 1024], f32, tag="s", name="pd")
                        for g in range(2):
                            nc.tensor.matmul(
                                pd[:, 0:512],
                                lhsT=(yT_sb[:, g, t0 * 128:(t0 + 1) * 128]),
                                rhs=(wo_sb[:, g, 0:512]),
                                start=(g == 0), stop=(g == 1),
                            )
                            nc.tensor.matmul(
                                pd[:, 512:1024],
                                lhsT=(yT_sb[:, g, t0 * 128:(t0 + 1) * 128]),
                                rhs=(wo_sb[:, g, 512:1024]),
                                start=(g == 0), stop=(g == 1),
                            )
                        nc.vector.tensor_copy(o_sb, pd)
                        nc.sync.dma_start(
                            out=out[t0 * 128:(t0 + 1) * 128, :], in_=o_sb
                        )

                run_pair(3, pre[(0, 3)])
                run_pair(2, pre[(0, 2)])
                emit_d(3)
                run_group(0, 1)
                emit_d(2)
                run_group(1, 1)
                run_group(0, 0)
                emit_d(1)
                run_group(1, 0)
                emit_d(0)

    nc.compile()
    return nc


def _host_consts():
    p = np.arange(128)[:, None]
    c = np.arange(128)[None, :]
    tri01 = (p >= c).astype(np.float16)
    return tri01


def make_in_maps(x, Wqkv, bqkv, Wo, bo):
    x = np.asarray(x, dtype=np.float32)
    Wqkv = np.asarray(Wqkv, dtype=np.float32)
    bqkv = np.asarray(bqkv, dtype=np.float32)
    Wo = np.asarray(Wo, dtype=np.float32)
    tri01 = _host_consts()
    xT = [np.ascontiguousarray(x[b].T).astype(np.float16) for b in range(B)]
    in_maps = []
    for core in range(N_CORES):
        b, hg = divmod(core, 4)
        s = HC * hg
        in_maps.append({
            "xbT": xT[b],
            # SCALE folded into wq (and bqs) so the q evacuation is a plain
            # bias add on DVE
            "wq": (np.ascontiguousarray(Wqkv[:, s:s + HC])
                   * np.float32(SCALE)).astype(np.float16),
            "wk": np.ascontiguousarray(Wqkv[:, C + s:C + s + HC]).astype(np.float16),
            "wv": np.ascontiguousarray(Wqkv[:, 2 * C + s:2 * C + s + HC]).astype(np.float16),
            "bqs": np.ascontiguousarray(bqkv[s:s + HC]) * np.float32(SCALE),
            "bk": np.ascontiguousarray(bqkv[C + s:C + s + HC]),
            "bvb": np.ascontiguousarray(
                np.broadcast_to(bqkv[2 * C + s:2 * C + s + HC], (128, HC))
            ),
            "wo": np.ascontiguousarray(Wo[s:s + HC, :]).astype(np.float16),
            "tri01": tri01,
        })
    return in_maps


def unshard(results, bo=None):
    out = np.empty((B, T, C), dtype=np.float32)
    for b in range(B):
        acc = results[4 * b]["out"].astype(np.float32)
        for hg in range(1, 4):
            acc = acc + results[4 * b + hg]["out"].astype(np.float32)
        if bo is not None:
            acc = acc + np.asarray(bo, dtype=np.float32)
        out[b] = acc
    return out


def get_nc():
    if "nc" not in _CACHE:
        _CACHE["nc"] = _build_nc()
    return _CACHE["nc"]


def kernel(x, Wqkv, bqkv, Wo, bo):
    from concourse.bass_utils import run_bass_kernel_spmd

    nc = get_nc()
    in_maps = make_in_maps(x, Wqkv, bqkv, Wo, bo)
    res = run_bass_kernel_spmd(nc, in_maps, list(range(N_CORES)))
    return unshard(res.results, bo=bo)
